# revision 1
# baseline (speedup 1.0000x reference)
"""Tensor-parallel FlashLlamaAttention kernel for 8 Trainium2 NeuronCores.

Sharding: each core owns 4 query heads (512 proj dims) and 1 kv head
(128 dims). Per-core device program computes qkv projection (+RoPE),
causal GQA attention and its o_proj partial product; the 8 partial
[2048, 4096] outputs are summed on the host (replaces the all-reduce).

Device-side layouts are all "feature on partitions" (transposed), so the
host wrapper pre-transposes hidden_states and the weight shards.
Matmul operands are float32r (full-rate fp32 PE mode). Rotate-half for
RoPE runs on the PE as a permutation matmul (DVE cannot move data
across partitions).
"""
import sys

sys.path.insert(0, "/opt/trn_rl_repo")

from contextlib import ExitStack

import numpy as np

import concourse.bass as bass
import concourse.bacc as bacc
import concourse.mybir as mybir
import concourse.tile as tile
from concourse.bass_utils import run_bass_kernel_spmd
from concourse.masks import make_identity

F32 = mybir.dt.float32
F32R = mybir.dt.float32r
EXP = mybir.ActivationFunctionType.Exp

P = 128          # partitions / head dim
T = 2048         # total tokens (B * S)
S = 1024         # seq len per batch
B = 2
HD = 4096        # hidden dim
NHL = 4          # local query heads per core
DQKV = NHL * P + P + P  # 768 local projection dims (4q + k + v)
SM = float(P) ** -0.5

N_CORES = 8


def build_nc():
    nc = bacc.Bacc("TRN2", target_bir_lowering=False, debug=False,
                   num_devices=N_CORES)

    hiddenT = nc.dram_tensor("hiddenT", [HD, T], F32R, kind="ExternalInput").ap()
    wqkvT = nc.dram_tensor("wqkvT", [HD, DQKV], F32R, kind="ExternalInput").ap()
    woT = nc.dram_tensor("woT", [NHL * P, HD], F32R, kind="ExternalInput").ap()
    cosF = nc.dram_tensor("cosF", [P, T], F32, kind="ExternalInput").ap()
    sinF = nc.dram_tensor("sinF", [P, T], F32, kind="ExternalInput").ap()
    out = nc.dram_tensor("out", [T, HD], F32, kind="ExternalOutput").ap()

    with tile.TileContext(nc) as tc, ExitStack() as stack:
        const = stack.enter_context(tc.tile_pool(name="const", bufs=1))
        ident = const.tile([P, P], F32)
        make_identity(nc, ident[:])
        # rotate-half permutation: perm[k, i] = 1 iff |k - i| == 64
        perm = const.tile([P, P], F32R)
        nc.vector.tensor_copy(perm[:, 0:64], ident[:, 64:128])
        nc.vector.tensor_copy(perm[:, 64:128], ident[:, 0:64])
        ones_f32 = const.tile([P, 1], F32)
        nc.vector.memset(ones_f32[:], 1.0)
        ones_k = const.tile([P, 1], F32R)
        nc.vector.tensor_copy(ones_k[:], ones_f32[:])
        # causal corner mask: keep (q=f) >= (k=p)
        mask = const.tile([P, P], F32)
        nc.gpsimd.memset(mask[:], 1.0)
        nc.gpsimd.affine_select(
            out=mask[:], in_=mask[:], compare_op=mybir.AluOpType.is_ge,
            fill=0.0, base=0, pattern=[[1, P]], channel_multiplier=-1)

        # long-lived activations
        qkv_pool = stack.enter_context(tc.tile_pool(name="qkv", bufs=1))
        qT = [qkv_pool.tile([P, T], F32R, tag=f"qT{h}", name=f"qT{h}")
              for h in range(NHL)]
        kT = qkv_pool.tile([P, T], F32R, tag="kT", name="kT")
        v_pool = stack.enter_context(tc.tile_pool(name="v", bufs=1))
        v_sb = v_pool.tile([P, T // P, P], F32R, tag="v_sb", name="v_sb")

        # ---------------- phase 1: qkv projection + rope -----------------
        with (
            tc.tile_pool(name="cs", bufs=1) as cs_pool,
            tc.tile_pool(name="w1", bufs=1) as w1_pool,
            tc.tile_pool(name="xt", bufs=12) as xt_pool,
            tc.tile_pool(name="rot", bufs=2) as rot_pool,
            tc.tile_pool(name="qraw", bufs=5) as qraw_pool,
            tc.tile_pool(name="vtmp", bufs=1) as vtmp_pool,
            tc.tile_pool(name="pps", bufs=1, space="PSUM") as proj_psum,
            tc.tile_pool(name="rps", bufs=2, space="PSUM") as rope_psum,
        ):
            wqkv_sb = w1_pool.tile([P, HD // P, DQKV], F32R)
            wqkvT_r = wqkvT.rearrange("(a p) j -> p a j", p=P)
            for g in range(8):
                nc.scalar.dma_start(wqkv_sb[:, 4 * g:4 * (g + 1), :],
                                    wqkvT_r[:, 4 * g:4 * (g + 1), :])
            cos_sb = cs_pool.tile([P, T], F32)
            sin_sb = cs_pool.tile([P, T], F32)
            nc.scalar.dma_start(cos_sb[:], cosF[:])
            nc.scalar.dma_start(sin_sb[:], sinF[:])
            vT_tmp = vtmp_pool.tile([P, T], F32, tag="vT", name="vT_tmp")

            for tc4 in range(T // 512):
                ts = slice(512 * tc4, 512 * (tc4 + 1))
                ps = [proj_psum.tile([P, 512], F32, tag=f"pps{ot}",
                                     name=f"pps{ot}_{tc4}")
                      for ot in range(6)]
                for ho in range(HD // P):
                    xt = xt_pool.tile([P, 512], F32R, tag="xt")
                    nc.sync.dma_start(xt[:], hiddenT[P * ho:P * (ho + 1), ts])
                    for ot in range(6):
                        nc.tensor.matmul(
                            ps[ot][:], wqkv_sb[:, ho, P * ot:P * (ot + 1)],
                            xt[:], start=(ho == 0), stop=(ho == HD // P - 1))
                # evacuate all 6 PSUM banks first (qraw copy + cos-mult
                # are each bank's only readers), so the next chunk's
                # matmuls and the attention pools unblock ASAP; the
                # rot/sin/add RoPE work runs afterwards off-PSUM
                qraws = []
                for idx, dst in enumerate(qT + [kT]):
                    qraw = qraw_pool.tile([P, 512], F32R, tag="qraw",
                                          name=f"qraw{idx}_{tc4}")
                    nc.any.tensor_copy(qraw[:], ps[idx][:])
                    qraws.append(qraw)
                    nc.vector.tensor_mul(out=dst[:, ts], in0=ps[idx][:],
                                         in1=cos_sb[:, ts])
                nc.vector.tensor_copy(vT_tmp[:, ts], ps[5][:])
                for idx, dst in enumerate(qT + [kT]):
                    rot_ps = rope_psum.tile([P, 512], F32, tag="rotp")
                    nc.tensor.matmul(rot_ps[:], perm[:], qraws[idx][:],
                                     start=True, stop=True)
                    rt = rot_pool.tile([P, 512], F32, tag="rot")
                    nc.vector.tensor_mul(out=rt[:], in0=rot_ps[:],
                                         in1=sin_sb[:, ts])
                    nc.vector.tensor_add(out=dst[:, ts], in0=dst[:, ts],
                                         in1=rt[:])
                # transpose this chunk of V: vT [j, t] -> v_sb [t, tchunk, j]
                for tt in range(4 * tc4, 4 * (tc4 + 1)):
                    trp = rope_psum.tile([P, P], F32, tag="rotp")
                    nc.tensor.transpose(trp[:], vT_tmp[:, P * tt:P * (tt + 1)],
                                        ident[:])
                    nc.any.tensor_copy(v_sb[:, tt, :], trp[:])

        # ----- long-lived attention output (allocated after phase 1 frees)
        at_pool = stack.enter_context(tc.tile_pool(name="at", bufs=1))
        ATn = [at_pool.tile([P, T], F32R, tag=f"ATn{h}", name=f"ATn{h}")
               for h in range(NHL)]
        outb_pool = stack.enter_context(tc.tile_pool(name="ob", bufs=2))

        with tc.tile_pool(name="w2", bufs=1) as w2_pool:
            # o_proj weights load early, overlaps attention compute
            wo_sb = w2_pool.tile([P, NHL, HD], F32R)
            woT_r = woT.rearrange("(a p) o -> p a o", p=P)
            for g in range(NHL):
                nc.scalar.dma_start(wo_sb[:, g, :], woT_r[:, g, :])

            # ---------------- phase 2: causal GQA attention --------------
            with (
                tc.tile_pool(name="pexp", bufs=6) as pexp_pool,
                tc.tile_pool(name="dens", bufs=3) as den_small,
                tc.tile_pool(name="rden", bufs=3) as rden_pool,
                tc.tile_pool(name="stp", bufs=2, space="PSUM") as st_psum,
                tc.tile_pool(name="atp", bufs=2, space="PSUM") as at_psum,
                tc.tile_pool(name="dnp", bufs=2, space="PSUM") as den_psum,
                tc.tile_pool(name="opp", bufs=2, space="PSUM") as op_psum,
            ):
                def oproj_tiles(t16_range):
                    """generator: one o_proj [t,512] tile per yield"""
                    for t16 in t16_range:
                        ob = outb_pool.tile([P, HD], F32, tag="ob")
                        for ot in range(HD // 512):
                            ps = op_psum.tile([P, 512], F32, tag="op")
                            for j in range(NHL):
                                nc.tensor.matmul(
                                    ps[:], ATn[j][:, P * t16:P * (t16 + 1)],
                                    wo_sb[:, j, 512 * ot:512 * (ot + 1)],
                                    start=(j == 0), stop=(j == NHL - 1))
                            nc.any.tensor_copy(
                                ob[:, 512 * ot:512 * (ot + 1)], ps[:])
                            if ot == 3:
                                nc.sync.dma_start(
                                    out[P * t16:P * (t16 + 1), 0:HD // 2],
                                    ob[:, 0:HD // 2])
                            yield
                        nc.sync.dma_start(
                            out[P * t16:P * (t16 + 1), HD // 2:],
                            ob[:, HD // 2:])
                def evac_group(b, h, qt, at_ps, den_ps):
                    """normalize+store one finished (b, h, qt) group"""
                    rrow = den_small.tile([1, 512], F32, tag="rrow")
                    nc.vector.reciprocal(rrow[:], den_ps[:])
                    rden = rden_pool.tile([P, 512], F32, tag="rden")
                    nc.gpsimd.partition_broadcast(rden[:], rrow[:])
                    nc.vector.tensor_mul(
                        out=ATn[h][:, S * b + 512 * qt:S * b + 512 * (qt + 1)],
                        in0=at_ps[:], in1=rden[:])

                def flush(item):
                    b, h, qt, a0, qo0, nk, px0, at_ps, den_ps = item
                    nc.tensor.matmul(
                        at_ps[:, qo0:], v_sb[:, (S // P) * b + a0, :],
                        px0[:, qo0:], start=(a0 == 0), stop=(a0 == nk - 1))
                    nc.tensor.matmul(
                        den_ps[:, qo0:], ones_k[:],
                        px0[:, qo0:], start=(a0 == 0), stop=(a0 == nk - 1))
                    if a0 == nk - 1:
                        evac_group(b, h, qt, at_ps, den_ps)

                pending = []       # score tiles awaiting their AV/den matmul

                def attn_tiles():
                    """generator: one score tile per yield (yields batch)"""
                    for b in range(B):
                        for h in range(NHL):
                            qTb = qT[h][:, S * b:S * (b + 1)]
                            kTb = kT[:, S * b:S * (b + 1)]
                            for qt in range(S // 512):
                                at_ps = at_psum.tile([P, 512], F32, tag="at")
                                den_ps = den_psum.tile([1, 512], F32,
                                                       tag="den")
                                nk = 4 * qt + 4
                                for a in range(nk):
                                    qoff = max(0, P * a - 512 * qt)
                                    st = st_psum.tile([P, 512], F32,
                                                      tag="st")
                                    nc.tensor.matmul(
                                        st[:, qoff:],
                                        kTb[:, P * a:P * (a + 1)],
                                        qTb[:, 512 * qt + qoff:
                                            512 * (qt + 1)],
                                        start=True, stop=True)
                                    pexp = pexp_pool.tile([P, 512], F32R,
                                                          tag="pexp")
                                    nc.scalar.activation(
                                        pexp[:, qoff:], st[:, qoff:], EXP,
                                        scale=SM)
                                    if P * a >= 512 * qt:
                                        nc.vector.tensor_mul(
                                            out=pexp[:, qoff:qoff + P],
                                            in0=pexp[:, qoff:qoff + P],
                                            in1=mask[:])
                                    pending.append((b, h, qt, a, qoff, nk,
                                                    pexp, at_ps, den_ps))
                                    if len(pending) == 4:
                                        flush(pending.pop(0))
                                    yield b

                # b=0 attention runs alone; during b=1 attention, o_proj of
                # the finished b=0 token chunks interleaves per head group
                DONE = object()
                ag = attn_tiles()
                og = oproj_tiles(range(T // P))
                state = next(ag)
                while state == 0:
                    state = next(ag, DONE)
                cnt = 1
                while state is not DONE:
                    state = next(ag, DONE)
                    cnt += 1
                    if cnt % 12 == 0:
                        for _ in range(16):
                            if next(og, DONE) is DONE:
                                break
                while pending:
                    flush(pending.pop(0))
                while next(og, DONE) is not DONE:
                    pass

    nc.compile()
    return nc


_NC = None


def _get_nc():
    global _NC
    if _NC is None:
        _NC = build_nc()
    return _NC


def make_in_maps(hidden_states, cos, sin, wq, wk, wv, wo):
    hidden_states = np.asarray(hidden_states, np.float32)
    cos = np.asarray(cos, np.float32)
    sin = np.asarray(sin, np.float32)
    wq = np.asarray(wq, np.float32)
    wk = np.asarray(wk, np.float32)
    wv = np.asarray(wv, np.float32)
    wo = np.asarray(wo, np.float32)

    HT = np.ascontiguousarray(hidden_states.T)
    cosT = cos.T
    sinT = sin.T
    cosF = np.ascontiguousarray(np.concatenate([cosT, cosT], 0))
    sinF = np.ascontiguousarray(np.concatenate([-sinT, sinT], 0))

    in_maps = []
    for c in range(N_CORES):
        wq_c = wq[NHL * P * c:NHL * P * (c + 1)]
        wk_c = wk[P * c:P * (c + 1)]
        wv_c = wv[P * c:P * (c + 1)]
        wqkvT = np.ascontiguousarray(np.concatenate([wq_c, wk_c, wv_c], 0).T)
        woT = np.ascontiguousarray(wo[:, NHL * P * c:NHL * P * (c + 1)].T)
        in_maps.append(dict(hiddenT=HT, wqkvT=wqkvT, woT=woT,
                            cosF=cosF, sinF=sinF))
    return in_maps


def kernel(hidden_states, cos, sin, wq, wk, wv, wo, batch, seq_len):
    assert int(batch) == B and int(seq_len) == S
    nc = _get_nc()
    in_maps = make_in_maps(hidden_states, cos, sin, wq, wk, wv, wo)
    res = run_bass_kernel_spmd(nc, in_maps, core_ids=list(range(N_CORES)))
    acc = res.results[0]["out"].astype(np.float32, copy=True)
    for c in range(1, N_CORES):
        acc += res.results[c]["out"]
    return acc



# revision 4
# speedup vs baseline: 1.1960x; 1.1960x over previous
"""Tensor-parallel FlashLlamaAttention kernel for 8 Trainium2 NeuronCores.

Sharding: each core owns 4 query heads (512 proj dims) and 1 kv head
(128 dims). Per-core device program computes qkv projection (+RoPE),
causal GQA attention and its o_proj partial product; the 8 partial
[2048, 4096] outputs are summed on the host (replaces the all-reduce).

v2: fully-fused single PE stream in bf16.
 - all matmul operands bf16 (1 cyc/row, same as f32r, but half the DMA
   and SBUF traffic); PSUM stays f32; host pre-casts inputs, output is
   written bf16 and summed in f32 on the host.
 - projection chunks, attention groups and o_proj blocks are emitted
   interleaved so the PE never idles across phase boundaries (idle gaps
   also reset the PE DVFS ramp).
 - RoPE rotate-half runs as a partition-crossing DMA (PSUM -> SBUF)
   instead of a PE permutation matmul.
 - V is projected directly in [token, dim] layout by using the hidden
   chunk as the matmul stationary, killing the PE transposes.
 - softmax denominator is accumulated broadcast across partitions via a
   ones[128,128] stationary and inverted with the fast approximate
   reciprocal (the exact DVE reciprocal costs 3.3us per row-tile and
   serialized the in-order DVE queue).
"""
import sys

sys.path.insert(0, "/opt/trn_rl_repo")

from contextlib import ExitStack

import numpy as np
import ml_dtypes

import concourse.bass as bass
import concourse.bacc as bacc
import concourse.mybir as mybir
import concourse.tile as tile
from concourse.bass_utils import run_bass_kernel_spmd

F32 = mybir.dt.float32
BF16 = mybir.dt.bfloat16
EXP = mybir.ActivationFunctionType.Exp

P = 128          # partitions / head dim
T = 2048         # total tokens (B * S)
S = 1024         # seq len per batch
B = 2
HD = 4096        # hidden dim
NHL = 4          # local query heads per core
DQKV = NHL * P + P + P  # 768 local projection dims (4q + k + v)
CH = 512         # token chunk for projection
KO = HD // P     # 32 contraction chunks
SM = float(P) ** -0.5

N_CORES = 8

_SENT = object()


def build_nc():
    nc = bacc.Bacc("TRN2", target_bir_lowering=False, debug=False,
                   num_devices=N_CORES)

    hiddenT = nc.dram_tensor("hiddenT", [HD, T], BF16, kind="ExternalInput").ap()
    wqkvT = nc.dram_tensor("wqkvT", [HD, DQKV], BF16, kind="ExternalInput").ap()
    woT = nc.dram_tensor("woT", [NHL * P, HD], BF16, kind="ExternalInput").ap()
    cosF = nc.dram_tensor("cosF", [P, T], BF16, kind="ExternalInput").ap()
    sinF = nc.dram_tensor("sinF", [P, T], BF16, kind="ExternalInput").ap()
    out = nc.dram_tensor("out", [T, HD], BF16, kind="ExternalOutput").ap()

    with tile.TileContext(nc) as tc, ExitStack() as stack:
        const = stack.enter_context(tc.tile_pool(name="const", bufs=1))
        ones_sb = const.tile([P, P], BF16)
        nc.vector.memset(ones_sb[:], 1.0)
        # causal corner mask: keep (q=f) >= (k=p)
        mask = const.tile([P, P], BF16)
        nc.gpsimd.memset(mask[:], 1.0)
        nc.gpsimd.affine_select(
            out=mask[:], in_=mask[:], compare_op=mybir.AluOpType.is_ge,
            fill=0.0, base=0, pattern=[[1, P]], channel_multiplier=-1)

        w1 = stack.enter_context(tc.tile_pool(name="w1", bufs=1))
        wqkv_sb = w1.tile([P, KO, DQKV], BF16)
        cs = stack.enter_context(tc.tile_pool(name="cs", bufs=1))
        cos_sb = cs.tile([P, T], BF16)
        sin_sb = cs.tile([P, T], BF16)
        qk = stack.enter_context(tc.tile_pool(name="qk", bufs=1))
        qT = [qk.tile([P, T], BF16, tag=f"qT{h}", name=f"qT{h}")
              for h in range(NHL)]
        kT = qk.tile([P, T], BF16, tag="kT", name="kT")
        vpool = stack.enter_context(tc.tile_pool(name="vp", bufs=1))
        v_sb = vpool.tile([P, T // P, P], BF16, tag="v_sb", name="v_sb")
        atn_pool = stack.enter_context(tc.tile_pool(name="atn", bufs=1))
        ATn = [atn_pool.tile([P, T], BF16, tag=f"ATn{h}", name=f"ATn{h}")
               for h in range(NHL)]
        w2 = stack.enter_context(tc.tile_pool(name="w2", bufs=1))
        wo_sb = w2.tile([P, NHL, HD], BF16)
        pexp_pool = stack.enter_context(tc.tile_pool(name="pexp", bufs=6))
        rden_pool = stack.enter_context(tc.tile_pool(name="rden", bufs=3))

        # weight / rotary table loads on the Act DMA ring; wqkv is issued in
        # 32 fine-grained slices so the first projection matmul starts early
        wqkvT_r = wqkvT.rearrange("(a p) j -> p a j", p=P)
        for g in range(KO):
            nc.scalar.dma_start(wqkv_sb[:, g, :], wqkvT_r[:, g, :])
        nc.scalar.dma_start(cos_sb[:], cosF[:])
        nc.scalar.dma_start(sin_sb[:], sinF[:])
        woT_r = woT.rearrange("(a p) o -> p a o", p=P)
        for g in range(NHL):
            nc.scalar.dma_start(wo_sb[:, g, :], woT_r[:, g, :])

        with (
            tc.tile_pool(name="stp", bufs=2, space="PSUM") as st_psum,
            tc.tile_pool(name="atp", bufs=2, space="PSUM") as at_psum,
            tc.tile_pool(name="dnp", bufs=2, space="PSUM") as den_psum,
        ):
            def attn_group(b, h, qt):
                """causal attention for one (batch, head, 512-q-block)"""
                qTb = qT[h][:, S * b:S * (b + 1)]
                kTb = kT[:, S * b:S * (b + 1)]
                at_ps = at_psum.tile([P, CH], F32, tag="at")
                den_ps = den_psum.tile([P, CH], F32, tag="den")
                nk = 4 * qt + 4
                pend = []

                def flush():
                    a, qo, px = pend.pop(0)
                    nc.tensor.matmul(at_ps[:, qo:], v_sb[:, (S // P) * b + a, :],
                                     px[:, qo:], start=(a == 0),
                                     stop=(a == nk - 1))
                    nc.tensor.matmul(den_ps[:, qo:], ones_sb[:], px[:, qo:],
                                     start=(a == 0), stop=(a == nk - 1))

                for a in range(nk):
                    qoff = max(0, P * a - CH * qt)
                    st = st_psum.tile([P, CH], F32, tag="st")
                    nc.tensor.matmul(st[:, qoff:], kTb[:, P * a:P * (a + 1)],
                                     qTb[:, CH * qt + qoff:CH * (qt + 1)],
                                     start=True, stop=True)
                    px = pexp_pool.tile([P, CH], BF16, tag="pexp")
                    nc.scalar.activation(px[:, qoff:], st[:, qoff:], EXP,
                                         scale=SM)
                    if P * a >= CH * qt:
                        nc.vector.tensor_mul(out=px[:, qoff:qoff + P],
                                             in0=px[:, qoff:qoff + P],
                                             in1=mask[:])
                    pend.append((a, qoff, px))
                    if len(pend) == 3:
                        flush()
                    yield
                while pend:
                    flush()
                rden = rden_pool.tile([P, CH], F32, tag="rden")
                nc.vector.reciprocal_approx_fast(out=rden[:], in_=den_ps[:])
                nc.vector.tensor_mul(
                    out=ATn[h][:, S * b + CH * qt:S * b + CH * (qt + 1)],
                    in0=at_ps[:], in1=rden[:])
                yield

            def attn_seq(b, qts):
                for qt in qts:
                    for h in range(NHL):
                        yield from attn_group(b, h, qt)

            def proj_chunk(c, pp, xt_pool, rot_pool, rt_pool):
                """qkv projection + rope for tokens [512c, 512c+512)"""
                ts = slice(CH * c, CH * (c + 1))
                xts = []
                for ho in range(KO):
                    t = xt_pool.tile([P, CH], BF16, tag="xt")
                    nc.sync.dma_start(t[:], hiddenT[P * ho:P * (ho + 1), ts])
                    xts.append(t)

                def qk_evac(idx, ps):
                    # dst = ps*cosF + rothalf(ps*sinG); sinG = sinF[rot(d)]
                    # so the partition-crossing move runs as a bf16 DMA
                    dst = qT[idx] if idx < NHL else kT
                    nc.vector.tensor_mul(out=dst[:, ts], in0=ps[:],
                                         in1=cos_sb[:, ts])
                    qs = rot_pool.tile([P, CH], BF16, tag="qs")
                    nc.vector.tensor_mul(out=qs[:], in0=ps[:],
                                         in1=sin_sb[:, ts])
                    rt = rt_pool.tile([P, CH], BF16, tag="rt")
                    nc.scalar.dma_start(rt[64:128, :], qs[0:64, :])
                    nc.scalar.dma_start(rt[0:64, :], qs[64:128, :])
                    nc.vector.tensor_add(out=dst[:, ts], in0=dst[:, ts],
                                         in1=rt[:])

                for pa in (0, 2):
                    psA = pp.tile([P, CH], F32, tag="pp")
                    psB = pp.tile([P, CH], F32, tag="pp")
                    for ho in range(KO):
                        nc.tensor.matmul(psA[:],
                                         wqkv_sb[:, ho, P * pa:P * (pa + 1)],
                                         xts[ho][:], start=(ho == 0),
                                         stop=(ho == KO - 1))
                        yield
                        nc.tensor.matmul(psB[:],
                                         wqkv_sb[:, ho, P * (pa + 1):P * (pa + 2)],
                                         xts[ho][:], start=(ho == 0),
                                         stop=(ho == KO - 1))
                        yield
                    qk_evac(pa, psA)
                    qk_evac(pa + 1, psB)
                psA = pp.tile([P, CH], F32, tag="pp")
                psB = pp.tile([P, CH], F32, tag="pp")
                for ho in range(KO):
                    nc.tensor.matmul(psA[:], wqkv_sb[:, ho, 4 * P:5 * P],
                                     xts[ho][:], start=(ho == 0),
                                     stop=(ho == KO - 1))
                    yield
                qk_evac(NHL, psA)
                # v projected with hidden as stationary -> [token, dim] psum
                for s4 in range(4):
                    for ho in range(KO):
                        nc.tensor.matmul(psB[:, P * s4:P * (s4 + 1)],
                                         xts[ho][:, P * s4:P * (s4 + 1)],
                                         wqkv_sb[:, ho, 5 * P:6 * P],
                                         start=(ho == 0), stop=(ho == KO - 1))
                        yield
                for s4 in range(4):
                    nc.vector.tensor_copy(v_sb[:, 4 * c + s4, :],
                                          psB[:, P * s4:P * (s4 + 1)])

            def chain(*gens):
                for g in gens:
                    yield from g

            def interleave(pg, ag, ratio):
                """1 attention yield : `ratio` proj yields; drain both"""
                done_p = done_a = False
                while not (done_p and done_a):
                    if not done_a and next(ag, _SENT) is _SENT:
                        done_a = True
                    for _ in range(ratio if not done_a else 1 << 30):
                        if next(pg, _SENT) is _SENT:
                            done_p = True
                            break
                    if done_p and not done_a:
                        for _ in ag:
                            pass
                        done_a = True

            with (
                tc.tile_pool(name="xt", bufs=40) as xt_pool,
                tc.tile_pool(name="rot", bufs=3) as rot_pool,
                tc.tile_pool(name="rt", bufs=3) as rt_pool,
                tc.tile_pool(name="pp", bufs=2, space="PSUM") as pp,
            ):
                args = (pp, xt_pool, rot_pool, rt_pool)
                # chunk 0 alone (nothing else is ready)
                for _ in proj_chunk(0, *args):
                    pass
                # chunk 1 (288y) x b0 qt0 attention (20y)
                interleave(proj_chunk(1, *args), attn_seq(0, [0]), 14)
                # chunks 2+3 (576y) x b0 qt1 attention (36y)
                interleave(chain(proj_chunk(2, *args), proj_chunk(3, *args)),
                           attn_seq(0, [1]), 16)

            with (
                tc.tile_pool(name="ob", bufs=2) as ob_pool,
                tc.tile_pool(name="opp", bufs=2, space="PSUM") as opp,
            ):
                def oproj_block(t16):
                    ob = ob_pool.tile([P, HD], BF16, tag="ob")
                    for ot in range(HD // CH):
                        ps = opp.tile([P, CH], F32, tag="op")
                        for j in range(NHL):
                            nc.tensor.matmul(ps[:],
                                             ATn[j][:, P * t16:P * (t16 + 1)],
                                             wo_sb[:, j, CH * ot:CH * (ot + 1)],
                                             start=(j == 0),
                                             stop=(j == NHL - 1))
                            yield
                        nc.any.tensor_copy(ob[:, CH * ot:CH * (ot + 1)], ps[:])
                        if ot == 3:
                            nc.sync.dma_start(
                                out[P * t16:P * (t16 + 1), 0:HD // 2],
                                ob[:, 0:HD // 2])
                    nc.sync.dma_start(out[P * t16:P * (t16 + 1), HD // 2:],
                                      ob[:, HD // 2:])

                # b1 attention x o_proj; blocks 0-7 (b0) ready at entry,
                # 8-11 after b1 qt0 evacs (attn yield 20), 12-15 at the end
                og = chain(*[oproj_block(t16) for t16 in range(T // P)])
                consumed = 0
                ready = 8
                ay = 0
                for _ in attn_seq(1, [0, 1]):
                    ay += 1
                    if ay >= 20:
                        ready = max(ready, 12)
                    cap = ready * 32
                    pulled = 0
                    while consumed < cap and pulled < 8:
                        if next(og, _SENT) is _SENT:
                            break
                        consumed += 1
                        pulled += 1
                while next(og, _SENT) is not _SENT:
                    pass

    nc.compile()
    return nc


_NC = None


def _get_nc():
    global _NC
    if _NC is None:
        _NC = build_nc()
    return _NC


def make_in_maps(hidden_states, cos, sin, wq, wk, wv, wo):
    bf = ml_dtypes.bfloat16
    hs = np.asarray(hidden_states, np.float32)
    HT = np.ascontiguousarray(hs.T).astype(bf)
    cosT = np.asarray(cos, np.float32).T
    sinT = np.asarray(sin, np.float32).T
    cosF = np.ascontiguousarray(np.concatenate([cosT, cosT], 0)).astype(bf)
    sinF = np.ascontiguousarray(np.concatenate([sinT, -sinT], 0)).astype(bf)
    wq = np.asarray(wq, np.float32)
    wk = np.asarray(wk, np.float32)
    wv = np.asarray(wv, np.float32)
    wo = np.asarray(wo, np.float32)

    in_maps = []
    for c in range(N_CORES):
        wq_c = wq[NHL * P * c:NHL * P * (c + 1)]
        wk_c = wk[P * c:P * (c + 1)]
        wv_c = wv[P * c:P * (c + 1)]
        wqkvT = np.ascontiguousarray(
            np.concatenate([wq_c, wk_c, wv_c], 0).T).astype(bf)
        woT = np.ascontiguousarray(
            wo[:, NHL * P * c:NHL * P * (c + 1)].T).astype(bf)
        in_maps.append(dict(hiddenT=HT, wqkvT=wqkvT, woT=woT,
                            cosF=cosF, sinF=sinF))
    return in_maps


def kernel(hidden_states, cos, sin, wq, wk, wv, wo, batch, seq_len):
    assert int(batch) == B and int(seq_len) == S
    nc = _get_nc()
    in_maps = make_in_maps(hidden_states, cos, sin, wq, wk, wv, wo)
    res = run_bass_kernel_spmd(nc, in_maps, core_ids=list(range(N_CORES)))
    acc = res.results[0]["out"].astype(np.float32)
    for c in range(1, N_CORES):
        acc += res.results[c]["out"].astype(np.float32)
    return acc
